# revision 15
# baseline (speedup 1.0000x reference)
"""Trainium2 Bass kernel: single attention head (B=8, S=2048, E=1024, H=64).

Sharding: data-parallel over batch -- each of the 8 NeuronCores computes one
batch element's full attention. No collectives; every HBM byte read once.

v2 design (streaming wavefront):
  - Inputs are cast to fp16 HOST-side and staged as [128, blk, chunk, 256]
    so every 512KB block DMA is one contiguous 4KB line per partition on the
    fast HWDGE (sync) queue. Halves HBM traffic vs f32 and frees GpSimd.
  - Q/K/V stream in 8 interleaved column-block rounds (q_b, k_b, v_b). The
    softmax exp -- the ScalarE floor at ~27us -- starts at ~3us and runs
    continuously instead of waiting for all projections.
  - Projections run as column-tiled concurrent pairs (2x PE): pass A puts
    q_b in BOTH partition halves (array cols 0-63 // 64-127); pass B puts
    k_b in half (b%2) and v_b in the other. This materializes q and k in
    both SBUF partition halves with zero cross-partition copies.
  - Scores are row-tiled 2x: even-parity key tiles use array rows 0-63
    (kt/qt low half), odd tiles rows 64-127, concurrently, into different
    PSUM banks. Scores stay transposed (keys on partitions) so softmax
    rowsums ride a ones-column in the AV stationary.
  - exp on ScalarE (scale=1/8 fused) writes fp16 P tiles; AV accumulates
    [v|1]^T @ P into one [65, 2048] PSUM right behind each exp cell.
  - Finalize: PE transposes 128-col chunks (PSUM regions reuse the proj
    bank via f16 bitcast), VectorE reciprocal + scale, batched f32 DMA out.

PSUM: 1 bank proj (A/B slices) + 3 banks score cells + 4 banks AV = 8.
"""

import numpy as np

import concourse.bass as bass  # noqa: F401  (engine namespaces live on nc)
import concourse.mybir as mybir
import concourse.tile as tile
from concourse import bacc
from concourse.bass_utils import run_bass_kernel_spmd
from concourse.masks import make_identity

B, S, E, H = 8, 2048, 1024, 64
EC = E // 128    # contraction chunks (128 partitions each)
NB = 8           # column-block rounds
CB = S // NB     # 256 columns per block
NT = S // 128    # key tiles
F16 = mybir.dt.float16
F32 = mybir.dt.float32

_CACHE = {}


def _build_nc():
    nc = bacc.Bacc(None)
    xq = nc.declare_dram_parameter("xq", [128, NB, EC, CB], F16, isOutput=False)
    xk = nc.declare_dram_parameter("xk", [128, NB, EC, CB], F16, isOutput=False)
    xv = nc.declare_dram_parameter("xv", [128, NB, EC, CB], F16, isOutput=False)
    wq = nc.declare_dram_parameter("wq", [128, EC, H], F16, isOutput=False)
    wk = nc.declare_dram_parameter("wk", [128, EC, H], F16, isOutput=False)
    wv = nc.declare_dram_parameter("wv", [128, EC, H], F16, isOutput=False)
    bq = nc.declare_dram_parameter("bq", [128, 1], F32, isOutput=False)
    bv = nc.declare_dram_parameter("bv", [128, 1], F32, isOutput=False)
    out = nc.declare_dram_parameter("out", [S, H], F32, isOutput=True)

    Exp = mybir.ActivationFunctionType.Exp

    with tile.TileContext(nc) as tc:
        with tc.tile_pool(name="const", bufs=1) as const, \
             tc.tile_pool(name="xqp", bufs=4) as xqp, \
             tc.tile_pool(name="xkp", bufs=4) as xkp, \
             tc.tile_pool(name="xvp", bufs=4) as xvp, \
             tc.tile_pool(name="vtp", bufs=2) as vtp, \
             tc.tile_pool(name="p5sb", bufs=2) as p5sb, \
             tc.tile_pool(name="pjp", bufs=1, space="PSUM") as pjp, \
             tc.tile_pool(name="scp", bufs=3, space="PSUM") as scp, \
             tc.tile_pool(name="oap", bufs=1, space="PSUM") as oap:

            # ---- constants ----
            wts = {}
            for nm, dram in (("q", wq), ("k", wk), ("v", wv)):
                wt = const.tile([128, EC, H], F16, name=f"w{nm}")
                nc.sync.dma_start(out=wt[:], in_=dram[:])
                wts[nm] = wt
            bq_t = const.tile([128, 1], F32, name="bq_t")
            nc.sync.dma_start(out=bq_t[:], in_=bq[:])
            bv_t = const.tile([128, 1], F32, name="bv_t")
            nc.sync.dma_start(out=bv_t[:], in_=bv[:])

            qt = const.tile([128, S], F16, name="qt")       # q^T in BOTH halves
            kt = const.tile([128, S], F16, name="kt")       # k^T: half (b%2) per block
            vaug = const.tile([128, NT, 80], F16, name="vaug")
            ptall = const.tile([128, NT, S], F16, name="ptall")  # exp(S^T) tiles
            oasb = const.tile([65, S], F16, name="oasb")
            ident = const.tile([128, 128], F16, name="ident")
            osb_all = const.tile([128, NT, H], F32, name="osb_all")

            make_identity(nc, ident[:])
            nc.vector.memset(vaug[:, :, 64], 1.0)

            # ---- input block DMAs (sync HWDGE ring, FIFO, prefetch depth 3)
            xqts, xkts, xvts = [], [], []

            def fetch_round(b):
                xkt = xkp.tile([128, EC, CB], F16, tag="xk", name=f"xkt{b}")
                nc.sync.dma_start(out=xkt[:], in_=xk[:, b])
                xkts.append(xkt)
                xvt = xvp.tile([128, EC, CB], F16, tag="xv", name=f"xvt{b}")
                nc.sync.dma_start(out=xvt[:], in_=xv[:, b])
                xvts.append(xvt)
                xqt = xqp.tile([128, EC, CB], F16, tag="xq", name=f"xqt{b}")
                nc.sync.dma_start(out=xqt[:], in_=xq[:, b])
                xqts.append(xqt)

            for b in range(3):
                fetch_round(b)

            work = pjp.tile([128, 2 * CB], F32, name="work")  # proj psum: A | B
            oa = oap.tile([65, S], F32, name="oa")            # AV accumulator

            # AV accumulation groups are PSUM-BANK granular: each oa bank
            # holds two 256-col columns; start on the bank's first MM only.
            av_bank_count = [0] * (NB // 2)

            def emit_cell(kb, qb):
                """Scores + exp + AV for 256 keys (tiles 2kb, 2kb+1) x 256 q."""
                g = (kb % 2) * 64
                cell = scp.tile([128, 2, CB], F32, tag="sc", name=f"sc{kb}_{qb}")
                for ti in range(2):
                    t = 2 * kb + ti
                    nc.tensor.matmul(
                        cell[:, ti, :],
                        kt[g:g + 64, t * 128:(t + 1) * 128],
                        qt[g:g + 64, qb * CB:(qb + 1) * CB],
                        start=True, stop=True, skip_group_check=True)
                nc.scalar.activation(
                    ptall[:, 2 * kb:2 * kb + 2, qb * CB:(qb + 1) * CB],
                    cell[:], Exp, scale=0.125)
                bank = qb // 2
                for ti in range(2):
                    t = 2 * kb + ti
                    cnt = av_bank_count[bank]
                    nc.tensor.matmul(
                        oa[:, qb * CB:(qb + 1) * CB],
                        vaug[:, t, 0:65],
                        ptall[:, t, qb * CB:(qb + 1) * CB],
                        start=(cnt == 0), stop=(cnt == 2 * NT - 1),
                        skip_group_check=True)
                    av_bank_count[bank] = cnt + 1

            for b in range(NB):
                bcols = slice(b * CB, (b + 1) * CB)
                if b + 3 < NB:
                    fetch_round(b + 3)

                # ---- pass B first: k_b in half (b%2), v_b in the other
                kh = (b % 2) * 64
                vh = 64 - kh
                for c in range(EC):
                    nc.tensor.matmul(
                        work[kh:kh + 64, CB:2 * CB], wts["k"][:, c, :],
                        xkts[b][:, c, :],
                        start=(c == 0), stop=(c == EC - 1),
                        skip_group_check=True)
                    nc.tensor.matmul(
                        work[vh:vh + 64, CB:2 * CB], wts["v"][:, c, :],
                        xvts[b][:, c, :],
                        start=(c == 0), stop=(c == EC - 1),
                        skip_group_check=True)
                nc.vector.tensor_copy(
                    kt[kh:kh + 64, bcols], work[kh:kh + 64, CB:2 * CB])
                vtb = vtp.tile([128, CB], F16, tag="vt", name=f"vtb{b}")
                nc.vector.tensor_scalar_add(
                    vtb[vh:vh + 64, :], work[vh:vh + 64, CB:2 * CB],
                    bv_t[vh:vh + 64])
                nc.sync.dma_start_transpose(
                    vaug[:, 2 * b:2 * b + 2, 0:64], vtb[vh:vh + 64, :])

                # row cells (kb=b, qb<b): scores vs already-built q columns
                for qb in range(b):
                    emit_cell(b, qb)

                # ---- pass A: q_b into both halves (col groups 0-63 | 64-127)
                for c in range(EC):
                    nc.tensor.matmul(
                        work[0:64, 0:CB], wts["q"][:, c, :], xqts[b][:, c, :],
                        start=(c == 0), stop=(c == EC - 1),
                        skip_group_check=True)
                    nc.tensor.matmul(
                        work[64:128, 0:CB], wts["q"][:, c, :], xqts[b][:, c, :],
                        start=(c == 0), stop=(c == EC - 1),
                        skip_group_check=True)
                nc.vector.tensor_scalar_add(qt[:, bcols], work[:, 0:CB], bq_t[:])

                # column cells (kb<=b, qb=b): parity alternates with kb
                for kb in range(b + 1):
                    emit_cell(kb, b)

            # ---- finalize: transpose, normalize, store ----
            out_r = out[:].rearrange("(t p) h -> p t h", p=128)
            for cq in range(4):
                nc.vector.tensor_copy(
                    oasb[:, cq * 512:(cq + 1) * 512],
                    oa[:, cq * 512:(cq + 1) * 512])
                for jj in range(4):
                    j = cq * 4 + jj
                    trt = scp.tile([128, 66], F16, tag="sc", name=f"tr{j}")
                    tr = trt[:, 0:65]
                    nc.tensor.transpose(
                        tr, oasb[:, j * 128:(j + 1) * 128], ident[0:65, 0:65])
                    rc = p5sb.tile([128, 1], F32, tag="rc", name=f"rc{j}")
                    nc.vector.reciprocal(rc[:], tr[:, 64:65])
                    nc.vector.tensor_scalar(
                        osb_all[:, j, :], tr[:, 0:64], rc[:], None,
                        op0=mybir.AluOpType.mult)
                nc.scalar.dma_start(
                    out=out_r[:, cq * 4:(cq + 1) * 4, :],
                    in_=osb_all[:, cq * 4:(cq + 1) * 4, :])

    nc.finalize()
    return nc


def get_nc():
    if "nc" not in _CACHE:
        _CACHE["nc"] = _build_nc()
    return _CACHE["nc"]


def _stage_x(x):
    # [S, E] f32 -> [128, NB, EC, CB] f16 with [p, b, c, s] = x[b*CB+s, c*128+p]
    xt = np.ascontiguousarray(x.T.astype(np.float16))          # [E, S]
    xt = xt.reshape(EC, 128, NB, CB).transpose(1, 2, 0, 3)     # [p, b, c, s]
    return np.ascontiguousarray(xt)


def make_in_maps(inputs):
    q = np.asarray(inputs["query"], np.float32)
    k = np.asarray(inputs["key_"], np.float32)
    v = np.asarray(inputs["value"], np.float32)
    wmats = {}
    for nm, key in (("wq", "Wq"), ("wk", "Wk"), ("wv", "Wv")):
        w = np.asarray(inputs[key], np.float32).astype(np.float16)  # [E, H]
        wmats[nm] = np.ascontiguousarray(
            w.reshape(EC, 128, H).transpose(1, 0, 2))               # [128, EC, H]
    bq = np.asarray(inputs["bq"], np.float32).reshape(H, 1)
    bv = np.asarray(inputs["bv"], np.float32).reshape(H, 1)
    bq_d = np.ascontiguousarray(np.tile(bq, (2, 1)))                # [128, 1]
    bv_d = np.ascontiguousarray(np.tile(bv, (2, 1)))
    in_maps = []
    for b in range(B):
        in_maps.append({
            "xq": _stage_x(q[b]),
            "xk": _stage_x(k[b]),
            "xv": _stage_x(v[b]),
            "wq": wmats["wq"], "wk": wmats["wk"], "wv": wmats["wv"],
            "bq": bq_d, "bv": bv_d,
        })
    return in_maps


def kernel(**inputs):
    nc = get_nc()
    in_maps = make_in_maps(inputs)
    res = run_bass_kernel_spmd(nc, in_maps, list(range(B)))
    return np.stack([res.results[b]["out"] for b in range(B)], axis=0)
